# revision 15
# baseline (speedup 1.0000x reference)
"""Trainium2 Bass kernel for DocumentBertScoringLoss (B=8192), v3.

loss = MSE + margin-ranking + (1 - cosine), shape-(1,) fp32.

Margin-ranking reduces to S = sum over unordered pairs {i<j} of
min(|p_i - p_j|, 0.1); the hinge sum = 0.1*B^2 - 2*S (diagonal
included).  Coverage: global chunks 0..63 of 128 rows; core k owns
chunk 8s+k at slot s, so slot = "class" of 8 chunks spread across
cores and the SPMD program hardcodes per-slot column ranges:

 - within-class: slot s covers cols [1024s,1024(s+1)) with one-sided
   clamp h = min(max(x,s1),s1+0.1) (PE ones-matmul reduce; corrected
   by 1024*sum(p_rows) on the host).
 - cross-class: a near-regular tournament (s covers s+1..s+3, +s+4
   for s<4) assigns each unordered class pair to one slot; those
   columns need min(|d|,0.1), computed per column range by one of:
     T-b: ACT Abs(x + (-s1)) -> |d| tile; DVE TSP(min 0.1, accum) sum.
     T-f: clamp-pair h1 = clamp-chain on X, h2 = clamp-chain on NEGX
          (host-negated fp16), both PE-reduced; h1+h2 sums telescope
          to min(|d|,0.1) with the s1 corrections cancelling exactly.
     T-a: DVE solo 3-pass (sub; STT max(-d,d); min+accum).
   (The 2-pass chained sub/abs_max TSP is rejected by the walrus
   backend, so |d| is produced via ACT Abs or the STT max(-d,d).)

Per-engine split tuned against the TimelineSim cost model: ACT runs
the Abs pieces, DVE runs min-passes + pairs + solo + 2 within clamps,
Pool runs 6 within clamps + the MSE/cosine dot products + final
partition reductions, PE reduces all clamp columns into one PSUM
accumulation group.  Final assembly happens on the HOST from an
8-float per-core output vector.
"""

import numpy as np

import concourse.bass as bass
import concourse.bacc as bacc
import concourse.mybir as mybir
from concourse.bass_utils import run_bass_kernel_spmd
from concourse.tile import TileContext
from concourse.alu_op_type import AluOpType

B = 8192
NCORES = 8
SLOTS = 8
MR_BIAS = 0.1
COS_EPS = 1e-8

F32 = mybir.dt.float32
F16 = mybir.dt.float16

# ACT Abs pieces (slot, lo, hi); DVE pairs each with a min+accum pass.
ACT_PIECES = [
    (6, 0, 2048),
    (7, 0, 2048),
    (1, 2048, 6144),
    (2, 3072, 5120),
    (0, 3072, 5120),
    (4, 5120, 6144),
    (4, 6144, 7168),
]
# clamp-pair pieces (slot, lo, hi, reduce): h1 from X, h2 from NEGX.
# reduce "pe" -> ones-matmul into psum_w; "dve" -> TSP add+accum passes.
PAIR_PIECES = [
    (5, 0, 1024, "dve"),
    (0, 1024, 3072, "pe"),
    (7, 2048, 3072, "dve"),
    (3, 4096, 6144, "pe"),
    (3, 6144, 8192, "pe"),
    (5, 6144, 8192, "pe"),
    (2, 5120, 7168, "pe"),
    (4, 7168, 8192, "mix"),
    (6, 7168, 8192, "mix"),
]
DVE_WITHIN = [7]
POOL_WITHIN = [0, 1, 2, 3, 4, 5, 6]

# negx ranges to broadcast (union of PAIR col ranges)
NEG_RANGES = [(0, 3072), (4096, 8192)]

N_WARM = 8
KPOS = (len(ACT_PIECES)
        + 2 * sum(1 for p in PAIR_PIECES if p[3] == "dve")
        + sum(1 for p in PAIR_PIECES if p[3] == "mix"))

_CACHED = {}


def _cov(s):
    c = [(s + 1) % 8, (s + 2) % 8, (s + 3) % 8]
    if s < 4:
        c.append((s + 4) % 8)
    return c


def _check_cover():
    for s in range(SLOTS):
        want = set()
        for b in _cov(s):
            want |= set(range(1024 * b, 1024 * (b + 1)))
        got = set()
        for (ss, lo, hi) in ACT_PIECES + [p[:3] for p in PAIR_PIECES]:
            if ss == s:
                r = set(range(lo, hi))
                assert not (got & r), f"overlap in slot {s}"
                got |= r
        assert got == want, f"slot {s} coverage mismatch"
    neg = set()
    for lo, hi in NEG_RANGES:
        neg |= set(range(lo, hi))
    for (ss, lo, hi, _r) in PAIR_PIECES:
        assert set(range(lo, hi)) <= neg, f"pair {ss} outside NEG_RANGES"


_check_cover()


def _build_nc():
    nc = bacc.Bacc("TRN2", target_bir_lowering=False, debug=False,
                   num_devices=NCORES)

    pred_d = nc.dram_tensor("predictions", [B], F32, kind="ExternalInput")
    p16_d = nc.dram_tensor("pred16", [B], F16, kind="ExternalInput")
    n16_d = nc.dram_tensor("pred16neg", [B], F16, kind="ExternalInput")
    g_d = nc.dram_tensor("correct_output", [B], F32, kind="ExternalInput")
    prow_d = nc.dram_tensor("p_rows_ext", [128, 4 * SLOTS], F32, kind="ExternalInput")
    out_d = nc.dram_tensor("out", [128, 20], F32, kind="ExternalOutput")

    AF = mybir.ActivationFunctionType
    pred_ap = pred_d[:]
    p16_ap = p16_d[:]
    n16_ap = n16_d[:]

    with TileContext(nc) as tc:
        with (
            tc.tile_pool(name="const", bufs=1) as cpool,
            tc.tile_pool(name="hbuf", bufs=2) as hpool,
            tc.tile_pool(name="psum", bufs=1, space="PSUM") as ppool,
        ):
            # ---- persistent tiles ----
            xbf = cpool.tile([128, B], F16, name="xbf")
            nbf = cpool.tile([128, B], F16, name="nbf")
            prow_ext = cpool.tile([128, 4 * SLOTS], F32, name="prow_ext")
            prow = prow_ext[:, 0:SLOTS]
            s2 = prow_ext[:, SLOTS:2 * SLOTS]
            neg_prow = prow_ext[:, 2 * SLOTS:3 * SLOTS]
            s2n = prow_ext[:, 3 * SLOTS:4 * SLOTS]
            pred32 = cpool.tile([128, B // 128], F32, name="pred32")
            g32 = cpool.tile([128, B // 128], F32, name="g32")
            d_tile = cpool.tile([128, B // 128], F32, name="d_tile")
            junk8 = cpool.tile([128, SLOTS], F16, name="junk8")
            out_big = cpool.tile([128, 20], F32, name="out_big")
            stacked = out_big[:, 0:5]
            acc_all = out_big[:, 5:5 + KPOS]
            ones_bf = cpool.tile([128, 1], F16, name="ones_bf")
            zeros1 = cpool.tile([128, 1], F32, name="zeros1")
            warm16 = cpool.tile([128, 1], F16, name="warm16")

            psum_w = ppool.tile([128, 512], F32, name="psum_w")

            # ---- DMAs (all HWDGE via SP; b0 first so DVE starts early,
            # negx ranges before pred32/g32) ----
            nc.sync.dma_start(
                xbf[:, 0:1024], p16_ap[0:1024].partition_broadcast(128))
            nc.gpsimd.dma_start(prow_ext, prow_d[:, :])
            for lo, hi in [(1024, 2048), (2048, 3072), (3072, 4096),
                           (4096, 5120), (5120, 6144), (6144, 7168),
                           (7168, 8192)]:
                nc.sync.dma_start(
                    xbf[:, lo:hi], p16_ap[lo:hi].partition_broadcast(128))
            for lo, hi in [(0, 2048), (2048, 3072), (4096, 6144),
                           (6144, 8192)]:
                nc.sync.dma_start(
                    nbf[:, lo:hi], n16_ap[lo:hi].partition_broadcast(128))
            nc.sync.dma_start(pred32, pred_ap.rearrange("(p c) -> p c", p=128))
            nc.sync.dma_start(g32, g_d[:].rearrange("(p c) -> p c", p=128))

            # ---- DVE prologue ----
            nc.vector.memset(ones_bf, 1.0)
            nc.vector.memset(zeros1, 0.0)
            nc.gpsimd.memset(out_big[:, 19:20], 0.0)

            # ---- ACT: dummy Abs so the table load runs at t~1us ----
            nc.scalar.activation(warm16, ones_bf, AF.Abs, bias=zeros1)

            # ---- PE psum group bookkeeping ----
            # Two PSUM groups: A (bulk, stops before the DVE accum tail
            # so Pool's 512-wide reduce hides), B (last tiles, 128-wide,
            # cheap to collapse at the very end).
            n_mm_a = (len(POOL_WITHIN) + len(DVE_WITHIN)) * 2 \
                + sum((hi - lo) // 512 * 2
                      for _, lo, hi, r in PAIR_PIECES if r == "pe") \
                + sum((hi - lo) // 512
                      for _, lo, hi, r in PAIR_PIECES if r == "mix") \
                + N_WARM
            mm_state = {"i": 0}

            def mm(h_slice):
                i = mm_state["i"]
                nc.tensor.matmul(
                    psum_w[0:1, 0:512], ones_bf, h_slice,
                    start=(i == 0), stop=(i == n_mm_a - 1),
                    skip_group_check=True,
                )
                mm_state["i"] = i + 1

            def reduce_h(h, w):
                for o in range(0, w, 512):
                    mm(h[:, o:o + 512])

            junk_w = cpool.tile([128, 512], F16, name="junk_w")
            nc.gpsimd.memset(junk_w, 0.0)
            for _ in range(N_WARM):
                mm(junk_w[:, 0:512])

            # ---- Pool: within-class clamps ----
            pool_h = {}
            for s in POOL_WITHIN:
                h = hpool.tile([128, 1024], F16, tag="h_pool", name="h_pool",
                               bufs=3)
                nc.gpsimd.tensor_scalar(
                    h, xbf[:, 1024 * s:1024 * (s + 1)],
                    prow[:, s:s + 1], s2[:, s:s + 1],
                    AluOpType.max, AluOpType.min,
                )
                pool_h[s] = h

            # ---- ACT: Abs pieces ----
            act_absd = []
            for (s, lo, hi) in ACT_PIECES:
                w = hi - lo
                absd = hpool.tile([128, 4096], F16, tag="absd_a",
                                  name="absd_a", bufs=6)
                nc.scalar.activation(
                    absd[:, 0:w], xbf[:, lo:hi], AF.Abs,
                    bias=neg_prow[:, s:s + 1],
                )
                act_absd.append(absd)

            # ---- DVE main stream ----
            acc_j = [0]

            def accum_of(t, w):
                j = acc_j[0]
                nc.vector.tensor_scalar(
                    t[:, 0:w], t[:, 0:w], 0.0, None,
                    AluOpType.add, AluOpType.add,
                    accum_out=acc_all[:, j:j + 1],
                )
                acc_j[0] = j + 1

            def min_accum(absd, w):
                j = acc_j[0]
                nc.vector.tensor_scalar(
                    absd[:, 0:w], absd[:, 0:w], MR_BIAS, None,
                    AluOpType.min, AluOpType.add,
                    accum_out=acc_all[:, j:j + 1],
                )
                acc_j[0] = j + 1

            def within_dve(s):
                h = hpool.tile([128, 1024], F16, tag="h_dve", name="h_dve",
                               bufs=2)
                nc.vector.tensor_scalar(
                    h, xbf[:, 1024 * s:1024 * (s + 1)],
                    prow[:, s:s + 1], s2[:, s:s + 1],
                    AluOpType.max, AluOpType.min,
                )
                return h

            def clamp1(src, sc1, sc2, s, lo, hi):
                w = hi - lo
                h = hpool.tile([128, 2048], F16, tag="h_pair", name="h_pair",
                               bufs=6)
                nc.vector.tensor_scalar(
                    h[:, 0:w], src[:, lo:hi],
                    sc1[:, s:s + 1], sc2[:, s:s + 1],
                    AluOpType.max, AluOpType.min,
                )
                return h

            def h1_of(p):
                s, lo, hi, _r = p
                return clamp1(xbf, prow, s2, s, lo, hi)

            def h2_of(p):
                s, lo, hi, _r = p
                return clamp1(nbf, neg_prow, s2n, s, lo, hi)

            P = PAIR_PIECES
            wof = lambda p: p[2] - p[1]
            # Production first, accum/min passes last.  X-side h1 tiles
            # early; h2 tiles once their negx range lands.
            h1_s5 = h1_of(P[0])               # b0
            h1_s0 = h1_of(P[1])               # b1,b2
            h1_s7 = h1_of(P[2])               # b2
            h1_p3 = h1_of(P[3])               # (3,[4096,6144)) b4,b5
            h1_p4 = h1_of(P[4])               # (3,[6144,8192)) b6,b7
            wh7 = within_dve(7)               # b7
            h1_p5 = h1_of(P[5])               # (5,[6144,8192))
            h1_p6 = h1_of(P[6])               # (2,[5120,7168))
            h1_p7 = h1_of(P[7])               # (4,[7168,8192))
            h1_p8 = h1_of(P[8])               # (6,[7168,8192))
            h2_s5 = h2_of(P[0])               # negx [0,1024)
            h2_s0 = h2_of(P[1])
            h2_s7 = h2_of(P[2])               # negx [2048,3072)
            h2_p3 = h2_of(P[3])               # negx [4096,6144)
            h2_p4 = h2_of(P[4])               # negx [6144,8192)
            h2_p5 = h2_of(P[5])
            h2_p6 = h2_of(P[6])
            h2_p7 = h2_of(P[7])
            h2_p8 = h2_of(P[8])
            # accum tail: pair accums first, ACT mins last (A6/A7 land
            # latest from ACT, so they close the stream)
            accum_of(h1_s5, wof(P[0]))
            accum_of(h2_s5, wof(P[0]))
            accum_of(h1_s7, wof(P[2]))
            accum_of(h2_s7, wof(P[2]))
            accum_of(h2_p7, wof(P[7]))
            accum_of(h2_p8, wof(P[8]))
            min_accum(act_absd[0], 2048)      # A1 (6,[0,2048))
            min_accum(act_absd[1], 2048)      # A2 (7,[0,2048))
            min_accum(act_absd[2], 4096)      # A3 (1,[2048,6144))
            min_accum(act_absd[3], 2048)      # A4 (2,[3072,5120))
            min_accum(act_absd[4], 2048)      # A5 (0,[3072,5120))
            min_accum(act_absd[5], 1024)      # A6 (4,[5120,6144))
            min_accum(act_absd[6], 1024)      # A7 (4,[6144,7168))

            # ---- PE: reduce clamp tiles, ordered by expected arrival ----
            reduce_h(h1_s0, wof(P[1]))        # DVE ~5.9
            reduce_h(pool_h[0], 1024)         # Pool ~5.4
            reduce_h(pool_h[1], 1024)         # ~6.9
            reduce_h(h1_p3, wof(P[3]))        # DVE ~8
            reduce_h(pool_h[2], 1024)         # ~8.4
            reduce_h(h1_p4, wof(P[4]))        # ~9.5
            reduce_h(pool_h[3], 1024)         # ~10
            reduce_h(wh7, 1024)               # ~10
            reduce_h(h1_p5, wof(P[5]))        # ~11
            reduce_h(pool_h[4], 1024)         # ~11.4
            reduce_h(h1_p6, wof(P[6]))        # ~12
            reduce_h(h2_s0, wof(P[1]))        # negx early ~12
            reduce_h(pool_h[5], 1024)         # ~13
            reduce_h(h1_p7, wof(P[7]))
            reduce_h(h1_p8, wof(P[8]))
            reduce_h(pool_h[6], 1024)         # ~14.5
            reduce_h(h2_p3, wof(P[3]))
            reduce_h(h2_p4, wof(P[4]))
            reduce_h(h2_p5, wof(P[5]))
            reduce_h(h2_p6, wof(P[6]))
            assert mm_state["i"] == n_mm_a, (mm_state["i"], n_mm_a)

            # ---- small terms: Pool builds (p-g)/(p+g); ACT accumulates
            # squares (Pool TSP+accum is rejected by the backend) ----
            nc.gpsimd.tensor_tensor(d_tile, pred32, g32, AluOpType.subtract)
            sp_tile = cpool.tile([128, B // 128], F32, name="sp_tile")
            nc.gpsimd.tensor_tensor(sp_tile, pred32, g32, AluOpType.add)
            junk64 = cpool.tile([128, B // 128], F16, name="junk64")
            nc.scalar.activation(
                junk8, prow, AF.Identity, bias=zeros1,
                accum_out=stacked[:, 0:1])
            nc.scalar.activation(
                junk64, d_tile, AF.Square, bias=zeros1,
                accum_out=stacked[:, 1:2])
            nc.scalar.activation(
                junk64, sp_tile, AF.Square, bias=zeros1,
                accum_out=stacked[:, 2:3])
            nc.scalar.activation(
                junk64, pred32, AF.Square, bias=zeros1,
                accum_out=stacked[:, 3:4])
            nc.scalar.activation(
                junk64, g32, AF.Square, bias=zeros1,
                accum_out=stacked[:, 4:5])

            # ---- tail: collapse psum_w on ACT into out_big[0,19]; the
            # rest of out_big (stacked | acc_all) exports raw and the host
            # does the final partition sums ----
            junk512 = cpool.tile([1, 512], F16, name="junk512")
            nc.scalar.activation(
                junk512, psum_w[0:1, 0:512], AF.Identity,
                bias=zeros1[0:1, :], accum_out=out_big[0:1, 19:20])

            nc.sync.dma_start(out_d[:, :], out_big)

    nc.compile()
    return nc


def kernel(predictions: np.ndarray, correct_output: np.ndarray) -> np.ndarray:
    pred = np.ascontiguousarray(np.asarray(predictions, dtype=np.float32))
    g = np.ascontiguousarray(np.asarray(correct_output, dtype=np.float32))

    if "nc" not in _CACHED:
        _CACHED["nc"] = _build_nc()
    nc = _CACHED["nc"]

    pr = pred.reshape(SLOTS, NCORES, 128)  # [s, k, p]
    p16 = pred.astype(np.float16)
    in_maps = []
    for k in range(NCORES):
        prow = np.ascontiguousarray(pr[:, k, :].T)  # [128, 8]
        prow_ext = np.concatenate(
            [prow, prow + np.float32(MR_BIAS), -prow,
             -prow + np.float32(MR_BIAS)], axis=1)
        in_maps.append({
            "predictions": pred,
            "pred16": p16,
            "pred16neg": -p16,
            "correct_output": g,
            "p_rows_ext": np.ascontiguousarray(prow_ext),
        })

    res = None
    last_exc = None
    for _attempt in range(3):
        try:
            res = run_bass_kernel_spmd(nc, in_maps, core_ids=list(range(NCORES)))
            break
        except Exception as e:  # transient NRT/axon device errors
            last_exc = e
            import time as _time
            _time.sleep(1.0)
    if res is None:
        raise last_exc

    S = 0.0
    p0 = None
    for r in res.results:
        o = np.asarray(r["out"], dtype=np.float64).reshape(128, 20)
        cols = o[:, :5 + KPOS].sum(axis=0)
        p_sum, sq, splus, pp, gg = cols[:5]
        pos = cols[5:5 + KPOS].sum()
        within = o[0, 19]
        S += pos + (within - 1024.0 * p_sum)
        if p0 is None:
            p0 = (sq, splus, pp, gg)
    sq, splus, pp, gg = p0
    dot = (splus - pp - gg) / 2.0
    Bf = float(B)
    mse = sq / Bf
    mr = MR_BIAS - 2.0 * S / (Bf * Bf)
    denom = max(np.sqrt(pp * gg), COS_EPS)
    sim = 1.0 - dot / denom
    return np.array([mse + mr + sim], dtype=np.float32)


if __name__ == "__main__":
    rng = np.random.default_rng(0)
    p = rng.standard_normal(B).astype(np.float32)
    g = rng.standard_normal(B).astype(np.float32)
    print(kernel(p, g))
